# revision 53
# baseline (speedup 1.0000x reference)
"""GAT (2-layer graph attention network) Trainium2 Bass kernel, v2.

N=4096 nodes, F=512 feats; layer1: 8 heads x 16 (ELU, concat); layer2:
1 head 128->16; log_softmax. Dense masked attention, row-parallel over
8 cores (core k owns rows [512k, 512k+512)).

v2 scheme ("mask-matmul"): the exp() of the attention scores is pushed
entirely to O(N) vectors. With s_ij = f1_i + f2_j and leaky(s) =
max(s, 0.2s):

  w_ij = exp(leaky(s)-K) = A_i*B_j       if s >= 0
                           C_i*D_j       if s <  0
  B_j = exp(f2_j - KB), D_j = exp(0.2(f2_j - KB))
  (KB is a static range bound -- any value works since the common scale
   cancels in the num/den ratio; only G_i = C_i/A_i
   = exp(-0.8(f1_i + KB)) survives.)

Per head the N^2 work is only:  ind = [s >= 0] (one 4x tensor_scalar on
DVE, or a sigmoid(4096 s) step on ACT) and one mask-mult
m = ind*adjT (DVE or GpSimd). PE then computes, accumulating over
j-tiles:
  M1|M2 = [B.Wh | D.Wh]^T @ m        (per head, 64-wide lhsT)
  M3    = [D.Wh stacked 4 heads]^T @ adjT   (head-shared, 2 matmuls/jt)
and the epilogue forms  h = (M1 + G.(M3-M2)) / (den-rows)  which equals
masked-softmax(leaky) @ Wh exactly (branch split is exact, K cancels).

lhsT blocks are padded to 32 partitions so all PSUM reads/writes sit on
the 0/32/64/96 partition-base grid (den row sits at block position 0 so
the epilogue needs no partition-16 extraction DMA).

Scheduling: indicator/mask units are split across DVE (4x tensor_scalar,
in-place mask-mult), ACT (sigmoid step) and GpSimd (self-contained
ind+mult, PSUM-free) by act_frac/pool_frac; adj arrives host-transposed
(4 strided loads instead of 32 DMA-xbar transposes, which serialized on
the shared HWDGE descriptor generator); per-head epilogues interleave
into the attention stream; ELU and the final log_softmax run as batched
wide ops to cut Tile sync-instruction count.
"""

import os
import sys
import contextlib

for _p in ("/opt/trn_rl_repo",):
    if _p not in sys.path and os.path.isdir(_p):
        sys.path.insert(0, _p)

import numpy as np
import ml_dtypes

import concourse.bass as bass
import concourse.bacc as bacc
import concourse.tile as tile
from concourse import mybir
from concourse.bass_utils import run_bass_kernel_spmd

BF16 = ml_dtypes.bfloat16
ALPHA = 0.2

F = 512      # input features
H = 8        # heads (layer 1)
D = 16       # per-head dim
C = 16       # classes
P = 128      # partitions
NCORES = 8
E = D + 1    # Wh columns + ones column
W = 32       # padded lhsT block width (PSUM alignment grid)
SIGK = 4096.0  # sigmoid step steepness for ACT-variant indicator
KB1 = 32.0     # static f2 upper bound, layer 1 (actual max ~4.4)
KB2 = 16.0     # static f2o upper bound, layer 2 (actual max ~0.5)


def build_gat(n=4096, ncores=NCORES, dbg=False, no_collective=False,
              act_frac=0.58, pool_frac=0.22, jb=2, ppbufs=13, ttbufs=2,
              pool_ind=1):
    """Build the SPMD Bass program for one core (row-parallel)."""
    R = n // ncores          # rows per core
    IC = R // P              # i-blocks per core
    JT = n // P              # j-tiles (partition tiles of full node dim)
    FC = F // P              # f chunks
    HD = H * D               # 128
    JB = jb                  # j-tiles per elementwise batch
    NB = JT // JB
    assert R % P == 0 and JT % JB == 0

    fp32 = mybir.dt.float32
    bf16 = mybir.dt.bfloat16

    nc = bacc.Bacc("TRN2", target_bir_lowering=False, debug=dbg,
                   num_devices=ncores)

    xT = nc.dram_tensor("xT", [F, n], bf16, kind="ExternalInput").ap()
    xTm = nc.dram_tensor("xTm", [F, R], bf16, kind="ExternalInput").ap()
    adjm = nc.dram_tensor("adjm", [n, R], bf16, kind="ExternalInput").ap()
    W1a = nc.dram_tensor("W1a", [F, HD], bf16, kind="ExternalInput").ap()
    w1c = nc.dram_tensor("w1c", [F, H], bf16, kind="ExternalInput").ap()
    w2c = nc.dram_tensor("w2c", [F, H], bf16, kind="ExternalInput").ap()
    WoA = nc.dram_tensor("WoA", [HD, C + 1], bf16, kind="ExternalInput").ap()
    w1o = nc.dram_tensor("w1o", [HD, 1], bf16, kind="ExternalInput").ap()
    identf = nc.dram_tensor("identf", [P, P], fp32, kind="ExternalInput").ap()
    out = nc.dram_tensor("out", [R, C], fp32, kind="ExternalOutput").ap()

    AF = mybir.ActivationFunctionType
    ALU = mybir.AluOpType
    AX = mybir.AxisListType

    # per-batch variant assignment: units = L1 (h,b) + L2 (b)
    n_units = H * NB + NB
    act_units = set()
    acc = 0.0
    for u in range(n_units):
        acc += act_frac
        if acc >= 1.0:
            acc -= 1.0
            act_units.add(u)
    pool_units = set()
    acc = 0.0
    for u in range(n_units):
        acc += pool_frac
        if acc >= 1.0:
            acc -= 1.0
            pool_units.add(u)

    def bview(ap, insert_at, count):
        """AP copy with a stride-0 dim of `count` inserted at position."""
        dims = [list(d) for d in ap.ap]
        dims = dims[:insert_at] + [[0, count]] + dims[insert_at:]
        return bass.AP(ap.tensor, ap.offset, dims)

    with tile.TileContext(nc) as tc, contextlib.ExitStack() as ctx:
        big = ctx.enter_context(tc.tile_pool(name="big", bufs=1))
        consts = ctx.enter_context(tc.tile_pool(name="consts", bufs=1))
        work = ctx.enter_context(tc.tile_pool(name="work", bufs=2))
        work1 = ctx.enter_context(tc.tile_pool(name="work1", bufs=1))
        sc_t = ctx.enter_context(tc.tile_pool(name="sc_t", bufs=ttbufs))
        sc_p = ctx.enter_context(tc.tile_pool(name="sc_p", bufs=ppbufs))
        psA = ctx.enter_context(tc.tile_pool(name="psA", bufs=2, space="PSUM"))
        psATT = ctx.enter_context(
            tc.tile_pool(name="psATT", bufs=1, space="PSUM"))
        dram = ctx.enter_context(tc.tile_pool(name="dram", bufs=1,
                                              space="DRAM"))

        # ---- const / persistent loads ----
        # small consts first so phase-2 f1 can start immediately
        xTm_sb = consts.tile([P, FC, R], bf16)
        nc.sync.dma_start(xTm_sb[:], xTm.rearrange("(c p) n -> p c n", p=P))
        w1c_sb = consts.tile([P, FC, H], bf16)
        nc.sync.dma_start(w1c_sb[:], w1c.rearrange("(c p) n -> p c n", p=P))
        w2c_sb = consts.tile([P, FC, H], bf16)
        nc.sync.dma_start(w2c_sb[:], w2c.rearrange("(c p) n -> p c n", p=P))
        W1a_sb = consts.tile([P, FC, HD], bf16)
        nc.sync.dma_start(W1a_sb[:], W1a.rearrange("(c p) n -> p c n", p=P))
        WoA_sb = consts.tile([P, C + 1], bf16)
        nc.sync.dma_start(WoA_sb[:], WoA)
        w1o_sb = consts.tile([P, 1], bf16)
        nc.sync.dma_start(w1o_sb[:], w1o)
        identf_sb = consts.tile([P, P], fp32)
        nc.sync.dma_start(identf_sb[:], identf)
        # xT is dead once phase 2's matmuls finish; wpack reuses its slot.
        # Loaded in node-range chunks so phase-2 PE work streams with DMA.
        xT_sb = big.tile([P, FC, n], bf16, tag="bigslot")
        NQ = 8
        nqs = n // NQ
        for q in range(NQ):
            nc.sync.dma_start(
                xT_sb[:, :, q * nqs:(q + 1) * nqs],
                xT.rearrange("(c p) n -> p c n",
                             p=P)[:, :, q * nqs:(q + 1) * nqs])

        # persistent intermediates
        whaug = big.tile([P, JT, H, E], bf16)      # [j%P, jt, h, (d|ones)]
        f1b_all = big.tile([P, H, R], bf16)        # f1[i] bcast on partitions
        f2col_sb = big.tile([P, JT, H], fp32)      # f2[j] per-partition
        f2c4k = big.tile([P, JT, H], fp32)         # 4096*f2 (sigmoid bias)
        bdcol = big.tile([P, JT, H, 2], fp32)      # B, D per-partition
        g_bc = big.tile([P, H, R], bf16)           # G = exp(-.8(f1+f2max))
        f1row_sb = consts.tile([H, R], fp32)
        f1row_bf = consts.tile([H, R], bf16)
        bias_all = consts.tile([P, H, 3], fp32)    # -f2max, -.2f2max, -.8f2max
        hT = big.tile([P, R], bf16)                # layer-1 out (elu,cat)^T
        onesb = consts.tile([1, P], bf16)
        nc.vector.memset(onesb[:], 1.0)
        onesf = consts.tile([1, P], fp32)
        nc.vector.memset(onesf[:], 1.0)

        # adjacency arrives host-transposed; own DMA queue (gpsimd)
        adjT = big.tile([P, JT, R], bf16)
        NAG = 4
        AG = JT // NAG
        for gch in range(NAG):
            nc.gpsimd.dma_start(
                adjT[:, gch * AG:(gch + 1) * AG, :],
                adjm.rearrange("(t p) r -> p t r",
                               p=P)[:, gch * AG:(gch + 1) * AG, :])

        # ---- phase 2: f1, f2max first (unblocks bias/bdcol), then f2/Wh ----
        pf1 = psA.tile([H, R], fp32, tag="ps")
        for fc in range(FC):
            nc.tensor.matmul(pf1[:], lhsT=w1c_sb[:, fc, :],
                             rhs=xTm_sb[:, fc, :],
                             start=(fc == 0), stop=(fc == FC - 1))
        nc.vector.tensor_copy(f1row_sb[:], pf1[:])
        nc.vector.tensor_copy(f1row_bf[:], f1row_sb[:])
        f1row_1 = consts.tile([1, H, R], bf16)
        nc.sync.dma_start(f1row_1[:], f1row_bf[:])

        # constant shift KB1 stands in for f2max (any upper bound works:
        # the common scale cancels in the num/den ratio; B ~ exp(f2-32)
        # ~ 1e-16 stays far above fp32/bf16 denormals)
        nc.vector.memset(bias_all[:, :, 0:1], -KB1)
        nc.vector.memset(bias_all[:, :, 1:2], -ALPHA * KB1)
        nc.vector.memset(bias_all[:, :, 2:3], -0.8 * KB1)
        # constant shift KB2 stands in for f2omax (see KB1 note)
        bias2 = consts.tile([P, 3], fp32)
        nc.vector.memset(bias2[:, 0:1], -KB2)
        nc.vector.memset(bias2[:, 1:2], -ALPHA * KB2)
        nc.vector.memset(bias2[:, 2:3], -0.8 * KB2)

        # f1 broadcast + G broadcast tiles
        for h in range(H):
            pb = psA.tile([P, R], fp32, tag="ps")
            nc.tensor.matmul(pb[:], lhsT=onesb[:],
                             rhs=f1row_1[0:1, h, :], start=True, stop=True)
            nc.scalar.copy(f1b_all[:, h, :], pb[:])
            nc.scalar.activation(g_bc[:, h, :], f1b_all[:, h, :], AF.Exp,
                                 bias=bias_all[:, h, 2:3], scale=-0.8)

        for b in range(JT // 4):
            pf2 = psA.tile([P, 4, H], fp32, tag="ps")
            for q in range(4):
                jt = b * 4 + q
                for fc in range(FC):
                    nc.tensor.matmul(
                        pf2[:, q, :],
                        lhsT=xT_sb[:, fc, jt * P:(jt + 1) * P],
                        rhs=w2c_sb[:, fc, :],
                        start=(fc == 0), stop=(fc == FC - 1))
            nc.vector.tensor_copy(f2col_sb[:, b * 4:(b + 1) * 4, :], pf2[:])
        nc.vector.tensor_scalar_mul(f2c4k[:], f2col_sb[:], SIGK)

        # B/D columns: bdcol[:,:,h,0] = exp(f2 - f2max); [...,1] = exp(.2(..))
        for h in range(H):
            nc.scalar.activation(bdcol[:, :, h, 0], f2col_sb[:, :, h], AF.Exp,
                                 bias=bias_all[:, h, 0:1], scale=1.0)
            nc.scalar.activation(bdcol[:, :, h, 1], f2col_sb[:, :, h], AF.Exp,
                                 bias=bias_all[:, h, 1:2], scale=ALPHA)

        # ---- phase 1: Wh_all (+ ones col); dstack fills stream per jt ----
        nc.vector.memset(whaug[:, :, :, 0:1], 1.0)
        # contiguous copy of the D-scaled blocks (matmul weight APs must be
        # single-free-dim; a strided 4-head view of wpack would not be)
        dstack = big.tile([P, JT, H, W], bf16)
        nc.gpsimd.memset(dstack[:, :, :, E:W], 0.0)
        for jt in range(JT):
            pw = psA.tile([P, HD], fp32, tag="ps")
            for fc in range(FC):
                nc.tensor.matmul(
                    pw[:],
                    lhsT=xT_sb[:, fc, jt * P:(jt + 1) * P],
                    rhs=W1a_sb[:, fc, :],
                    start=(fc == 0), stop=(fc == FC - 1))
            nc.scalar.copy(
                whaug[:, jt, :, 1:E],
                pw[:].rearrange("p (h d) -> p h d", d=D))
            d_ap = bdcol[:, jt, :, 1:2]
            in1d = bass.AP(d_ap.tensor, d_ap.offset,
                           [list(d) for d in d_ap.ap] + [[0, E]])
            nc.gpsimd.tensor_tensor(dstack[:, jt, :, 0:E],
                                    whaug[:, jt, :, :], in1d, op=ALU.mult)

        # wpack takes over xT's slot (first write gated on xT's last read);
        # fills are interleaved into head 0's attention loop below.
        wpack = big.tile([P, JT, H, 2, W], bf16, tag="bigslot")
        nc.gpsimd.memset(wpack[:, :, :, :, E:W], 0.0)

        def wpack_fill(jt):
            in0 = bview(whaug[:, jt, :, :], 2, 2)       # [P, H, 2*, E]
            b_ap = bdcol[:, jt, :, :]
            in1 = bass.AP(b_ap.tensor, b_ap.offset,
                          [list(d) for d in b_ap.ap] + [[0, E]])
            nc.vector.tensor_tensor(wpack[:, jt, :, :, 0:E], in0, in1,
                                    op=ALU.mult)

        # ---- phase 4: layer-1 attention ----
        # PSUM: 4 per-head banks (2 heads each at offsets 0/64), 2 stack banks
        pattb = [psATT.tile([P, R], fp32, tag=f"att{b}", name=f"pattb{b}")
                 for b in range(4)]
        pattS = [psATT.tile([P, R], fp32, tag=f"stk{g}", name=f"pattS{g}")
                 for g in range(2)]

        # head-shared stack first (independent of ind/mm): PE warmup
        for jt in range(JT):
            for g in range(2):
                nc.tensor.matmul(
                    pattS[g][:],
                    lhsT=dstack[:, jt, 4 * g:4 * g + 4, :].rearrange(
                        "p h w -> p (h w)"),
                    rhs=adjT[:, jt, :],
                    start=(jt == 0), stop=(jt == JT - 1))

        def stage_ind(unit, fb, fcol, f4kcol, jt0):
            """Indicator tiles for one batch (DVE 4x TS or ACT sigmoid)."""
            pp = sc_p.tile([P, JB, R], bf16, tag="pp")
            for q in range(JB):
                if unit in act_units:
                    nc.scalar.activation(pp[:, q, :], fb, AF.Sigmoid,
                                         bias=f4kcol(jt0 + q), scale=SIGK)
                elif pool_ind and unit in pool_units:
                    nc.gpsimd.tensor_scalar(pp[:, q, :], fb, fcol(jt0 + q),
                                            0.0, op0=ALU.add, op1=ALU.is_ge)
                else:
                    nc.vector.tensor_scalar(pp[:, q, :], fb, fcol(jt0 + q),
                                            0.0, op0=ALU.add, op1=ALU.is_ge)
            return pp

        def stage_mult(unit, pp, jt0):
            """Mask-mult, in place (halves tile chains and sync edges)."""
            eng = nc.gpsimd if unit in pool_units else nc.vector
            eng.tensor_tensor(pp[:], pp[:],
                              adjT[:, jt0:jt0 + JB, :], op=ALU.mult)
            return pp

        def stage_mm(mm, wtile, pt, po, jt0):
            for q in range(JB):
                jt = jt0 + q
                nc.tensor.matmul(
                    pt[po:po + 2 * W, :], lhsT=wtile(jt), rhs=mm[:, q, :],
                    start=(jt == 0), stop=(jt == JT - 1))

        # stack copies (PSUM -> SBUF) can run during head-0 attention
        s3c = [work1.tile([P, R], fp32, tag=f"s3c{g}", name=f"s3c{g}")
               for g in range(2)]
        for g in range(2):
            nc.scalar.copy(s3c[g][:], pattS[g][:])

        def emit_epilogue(h):
            """h = (M1 + G.(M3-M2)) / den, then ELU, into hT strip.
            M blocks are read straight out of PSUM by the DVE ops."""
            b, o = h // 2, 64 * (h % 2)
            g, so = h // 4, W * (h % 4)
            # gpsimd cannot access PSUM: only t3 (all-SBUF) may go there
            eng = nc.gpsimd if h % 3 == 2 else nc.vector
            u = work.tile([W, R], fp32, tag="u")
            nc.vector.tensor_tensor(u[:], pattb[b][o + W:o + 2 * W, :],
                                    s3c[g][so:so + W, :], op=ALU.subtract)
            t3 = work.tile([W, R], fp32, tag="t3")
            eng.tensor_tensor(t3[:], g_bc[0:W, h, :], u[:],
                              op=ALU.mult)
            num = work.tile([W, R], fp32, tag="num")
            nc.vector.tensor_tensor(num[:], pattb[b][o:o + W, :], t3[:],
                                    op=ALU.subtract)
            recip = work.tile([1, R], bf16, tag="recip")
            with nc.allow_low_precision(reason="bf16 1/den: 0.4% rel, "
                                        "within 2e-2 budget"):
                nc.vector.reciprocal(recip[:], num[0:1, :])
            prb = psA.tile([W, R], fp32, tag="ps")
            nc.tensor.matmul(prb[:], lhsT=onesb[0:1, 0:W], rhs=recip[:],
                             start=True, stop=True)
            hph = work.tile([W, R], fp32, tag="hph")
            nc.vector.tensor_tensor(hph[:], num[:], prb[:], op=ALU.mult)
            nc.sync.dma_start(hpre[h * D:(h + 1) * D, :], hph[1:E, :])

        hpre = big.tile([P, R], fp32)

        # software-pipelined issue: ind(k) | mult(k-1) | matmuls(k-2), so no
        # engine queues a cross-engine-dependent op ahead of ready work.
        units = []
        for h in range(H):
            for b in range(NB):
                units.append((h * NB + b, h, b * JB))
        state = {}
        for step in range(len(units) + 2):
            if step < len(units):
                unit, h, jt0 = units[step]
                if h == 0:
                    wpack_fill(jt0)
                    wpack_fill(jt0 + 1)
                pp = stage_ind(
                    unit, fb=f1b_all[:, h, :],
                    fcol=lambda jt: f2col_sb[:, jt, h:h + 1],
                    f4kcol=lambda jt: f2c4k[:, jt, h:h + 1],
                    jt0=jt0)
                state[step] = [unit, h, jt0, pp, None]
            if 1 <= step <= len(units):
                rec = state[step - 1]
                rec[4] = stage_mult(rec[0], rec[3], rec[2])
            if step >= 2:
                unit, h, jt0, pp, mm = state.pop(step - 2)
                stage_mm(mm,
                         wtile=lambda jt: wpack[:, jt, h, :, :].rearrange(
                             "p b w -> p (b w)"),
                         pt=pattb[h // 2], po=64 * (h % 2), jt0=jt0)
        for h in range(H):
            emit_epilogue(h)

        # ELU deferred to one block (keeps Exp out of the sigmoid-table span)
        etile = work1.tile([P, R], fp32, tag="etile")
        nc.scalar.activation(etile[:], hpre[:], AF.Exp, bias=0.0, scale=1.0)
        em = work1.tile([P, R], fp32, tag="em")
        nc.vector.tensor_scalar(em[:], etile[:], 1.0, 0.0,
                                op0=ALU.subtract, op1=ALU.min)
        nc.vector.tensor_tensor(hT[:], hpre[:], em[:], op=ALU.max)

        # ---- phase 5: layer 2 ----
        gsrc = dram.tile([R, C + 1], fp32)
        for icb in range(IC):
            pg = psA.tile([P, C + 1], fp32, tag="ps")
            nc.tensor.matmul(pg[:], lhsT=hT[:, icb * P:(icb + 1) * P],
                             rhs=WoA_sb[:], start=True, stop=True)
            gs = work.tile([P, C + 1], fp32, tag="gs")
            nc.vector.tensor_copy(gs[:], pg[:])
            nc.sync.dma_start(
                gsrc[:].rearrange("(c p) e -> p c e", p=P)[:, icb, :], gs[:])
        pf1o = psA.tile([1, R], fp32, tag="ps")
        nc.tensor.matmul(pf1o[:], lhsT=w1o_sb[:], rhs=hT[:],
                         start=True, stop=True)
        f1orow = consts.tile([1, R], fp32)
        nc.vector.tensor_copy(f1orow[:], pf1o[:])
        f1orow_bf = consts.tile([1, R], bf16)
        nc.vector.tensor_copy(f1orow_bf[:], f1orow[:])
        pf1ob = psA.tile([P, R], fp32, tag="ps")
        nc.tensor.matmul(pf1ob[:], lhsT=onesb[:], rhs=f1orow_bf[:],
                         start=True, stop=True)
        f1ob = big.tile([P, R], bf16)
        nc.scalar.copy(f1ob[:], pf1ob[:])

        gdst = dram.tile([n, C + 1], fp32)
        if no_collective:
            # timing-sim stand-in (TimelineSim can't model collectives)
            for k in range(ncores):
                nc.sync.dma_start(gdst[k * R:(k + 1) * R, :], gsrc[:])
        else:
            nc.gpsimd.collective_compute(
                "AllGather", ALU.bypass,
                replica_groups=[list(range(ncores))],
                ins=[gsrc.opt()], outs=[gdst.opt()])

        wh2aug = big.tile([P, JT, C + 1], bf16)
        g_sb = big.tile([P, JT, C + 1], fp32)
        nc.sync.dma_start(g_sb[:], gdst[:].rearrange("(t p) e -> p t e", p=P))
        nc.scalar.copy(wh2aug[:, :, 1:C + 1], g_sb[:, :, 0:C])
        nc.vector.memset(wh2aug[:, :, 0:1], 1.0)


        g2_bc = big.tile([P, R], bf16)
        nc.scalar.activation(g2_bc[:], f1ob[:], AF.Exp,
                             bias=bias2[:, 2:3], scale=-0.8)
        bd2col = big.tile([P, JT, 2], fp32)
        nc.scalar.activation(bd2col[:, :, 0], g_sb[:, :, C], AF.Exp,
                             bias=bias2[:, 0:1], scale=1.0)
        nc.scalar.activation(bd2col[:, :, 1], g_sb[:, :, C], AF.Exp,
                             bias=bias2[:, 1:2], scale=ALPHA)
        f2o4k = big.tile([P, JT], fp32)
        nc.vector.tensor_scalar_mul(f2o4k[:], g_sb[:, :, C], SIGK)

        wp2 = big.tile([P, JT, 2, W], bf16)
        nc.gpsimd.memset(wp2[:, :, :, E:W], 0.0)
        in0 = bview(wh2aug[:], 2, 2)
        b2_ap = bd2col[:]
        in1 = bass.AP(b2_ap.tensor, b2_ap.offset,
                      [list(d) for d in b2_ap.ap] + [[0, E]])
        nc.vector.tensor_tensor(wp2[:, :, :, 0:E], in0, in1, op=ALU.mult)

        patt2 = psATT.tile([2 * W, R], fp32, tag="att0")
        pattS2 = psATT.tile([W, R], fp32, tag="stk0")
        for jt in range(JT):
            nc.tensor.matmul(
                pattS2[:], lhsT=wp2[:, jt, 1, :], rhs=adjT[:, jt, :],
                start=(jt == 0), stop=(jt == JT - 1))
        state = {}
        for step in range(NB + 2):
            if step < NB:
                unit, jt0 = H * NB + step, step * JB
                pp = stage_ind(
                    unit, fb=f1ob[:],
                    fcol=lambda jt: g_sb[:, jt, C:C + 1],
                    f4kcol=lambda jt: f2o4k[:, jt:jt + 1],
                    jt0=jt0)
                state[step] = [unit, jt0, pp, None]
            if 1 <= step <= NB:
                rec = state[step - 1]
                rec[3] = stage_mult(rec[0], rec[2], rec[1])
            if step >= 2:
                unit, jt0, pp, mm = state.pop(step - 2)
                stage_mm(mm,
                         wtile=lambda jt: wp2[:, jt, :, :].rearrange(
                             "p b w -> p (b w)"),
                         pt=patt2, po=0, jt0=jt0)

        # ---- L2 epilogue + log_softmax ----
        s3c2 = work1.tile([W, R], fp32, tag="s3c2")
        nc.scalar.copy(s3c2[:], pattS2[:])
        u2 = work1.tile([W, R], fp32, tag="u2")
        nc.vector.tensor_tensor(u2[:], patt2[W:2 * W, :], s3c2[:],
                                op=ALU.subtract)
        t32 = work1.tile([W, R], fp32, tag="t32")
        nc.vector.tensor_tensor(t32[:], g2_bc[0:W, :], u2[:], op=ALU.mult)
        num2 = work1.tile([W, R], fp32, tag="num2")
        nc.vector.tensor_tensor(num2[:], patt2[0:W, :], t32[:],
                                op=ALU.subtract)

        # final log_softmax: per-icb transposes, then batched [P, IC, C] ops
        # (broadcast stride-0 views replace per-partition-scalar ops)
        posb = work1.tile([P, IC, C + 1], fp32, tag="posb")
        for icb in range(IC):
            po = psA.tile([P, C + 1], fp32, tag="ps")
            nc.tensor.transpose(po[:], num2[0:C + 1,
                                            icb * P:(icb + 1) * P],
                                identf_sb[0:C + 1, 0:C + 1])
            nc.scalar.copy(posb[:, icb, :], po[:])

        def cb(ap):  # broadcast a [P, IC, 1] column across C
            return bass.AP(ap.tensor, ap.offset,
                           [list(d) for d in ap.ap[:-1]] + [[0, C]])

        rc = work1.tile([P, IC, 1], fp32, tag="rc")
        nc.vector.reciprocal(rc[:], posb[:, :, 0:1])
        z = work1.tile([P, IC, C], fp32, tag="z")
        nc.vector.tensor_tensor(z[:], posb[:, :, 1:C + 1], cb(rc[:]),
                                op=ALU.mult)
        negmx = work1.tile([P, IC, 1], fp32, tag="negmx")
        nc.vector.tensor_reduce(negmx[:], z[:], axis=AX.X, op=ALU.max,
                                negate=True)
        zn = work1.tile([P, IC, C], fp32, tag="zn")
        nc.vector.tensor_tensor(zn[:], z[:], cb(negmx[:]), op=ALU.add)
        ez = work1.tile([P, IC, C], fp32, tag="ez")
        nc.scalar.activation(ez[:], zn[:], AF.Exp, bias=0.0, scale=1.0)
        sume = work1.tile([P, IC, 1], fp32, tag="sume")
        nc.vector.tensor_reduce(sume[:], ez[:], axis=AX.X, op=ALU.add)
        lns = work1.tile([P, IC, 1], fp32, tag="lns")
        nc.scalar.activation(lns[:], sume[:], AF.Ln, bias=0.0, scale=1.0)
        zo = work1.tile([P, IC, C], fp32, tag="zo")
        nc.vector.tensor_tensor(zo[:], zn[:], cb(lns[:]), op=ALU.subtract)
        nc.sync.dma_start(out.rearrange("(c p) e -> p c e", p=P), zo[:])

    nc.compile()
    return nc


def prep_inputs(x, adj, W1, a1, Wout, a_out, n=4096, ncores=NCORES):
    """Host-side prep: slice + transpose + bf16 cast + weight folds."""
    R = n // ncores
    x = np.asarray(x, np.float32)
    adj = np.asarray(adj)
    W1 = np.asarray(W1, np.float32)
    a1 = np.asarray(a1, np.float32)
    Wout = np.asarray(Wout, np.float32)
    a_out = np.asarray(a_out, np.float32)

    xT = np.ascontiguousarray(x.T).astype(BF16)
    W1a = np.ascontiguousarray(
        W1.transpose(1, 0, 2).reshape(F, H * D)).astype(BF16)
    w1c = np.ascontiguousarray(
        np.einsum("hfd,hd->fh", W1, a1[:, :D])).astype(BF16)
    w2c = np.ascontiguousarray(
        np.einsum("hfd,hd->fh", W1, a1[:, D:])).astype(BF16)
    w2o = Wout @ a_out[C:]
    WoA = np.ascontiguousarray(
        np.concatenate([Wout, w2o[:, None]], axis=1)).astype(BF16)
    w1o = np.ascontiguousarray((Wout @ a_out[:C])[:, None]).astype(BF16)
    identf = np.eye(P, dtype=np.float32)

    adj_bf = adj.astype(np.float32).astype(BF16)
    in_maps = []
    for k in range(ncores):
        rows = slice(k * R, (k + 1) * R)
        in_maps.append({
            "xT": xT,
            "xTm": np.ascontiguousarray(x[rows].T).astype(BF16),
            "adjm": np.ascontiguousarray(adj_bf[rows].T),
            "W1a": W1a, "w1c": w1c, "w2c": w2c,
            "WoA": WoA, "w1o": w1o,
            "identf": identf,
        })
    return in_maps


_cached = {}


def kernel(x, adj, W1, a1, Wout, a_out):
    n = x.shape[0]
    if n not in _cached:
        _cached[n] = build_gat(n=n)
    nc = _cached[n]
    in_maps = prep_inputs(x, adj, W1, a1, Wout, a_out, n=n)
    res = run_bass_kernel_spmd(nc, in_maps, core_ids=list(range(NCORES)))
    outs = [res.results[k]["out"] for k in range(NCORES)]
    return np.concatenate(outs, axis=0)
